# revision 4
# baseline (speedup 1.0000x reference)
"""DP-factored Trainium2 kernel for nn_Net_ht (no collectives).

Math: each HT block out[(d1 d2 d3)] = sum_{a,r1} u1[a,d1,r1] * (Q_r1 @ x[a,:,:])
with Q_r1[(d2 d3),(b c)] = sum_{r2,r4} (b2@b1)[r1,r2,r4] u2[..r2] u3[..r4]
composed on host.  Pure data parallel: z=32 batch rows per core, weights
(Q ~4MB/block + tiny u1-delta matrices) replicated; no collectives.

Per-core per-block pipeline (every PE contraction is partition-wise):
  X layout: ((b c) partition tiles, (z a) free, z-major)
  stage1:  t[r1][(m) part, (z a) free] = Q_r1_tiles . X        (matmul)
  tpose:   tT[(z a) part, (r1, m) free]                        (PE transpose)
  stage2:  y[(z8 d1) part, m free] per z-octet o = U1D . tT    (matmul; lhsT
           U1D[(z8 a),(z8' d1)] = u1 * delta_{z8 z8'}, host-built)
  tpose2:  X_next[(m=(d2 d3)) part, (z d1) free]               (PE transpose)
which restores the layout invariant for the next block.  fc6 on host.
"""
import os

import numpy as np
import concourse.bass as bass
from concourse import bacc
import concourse.mybir as mybir
from concourse.tile import TileContext
from concourse.bass_utils import run_bass_kernel_spmd
from concourse.masks import make_identity

NCORES = 8
ZC = 32              # batch rows per core
P = 128
R = 16
ZJ = 8               # z rows per output octet tile
DT_MODE = os.environ.get("HT_DT", "fp32")  # "fp32" (exact) or "tf32" (float32r, ~2x faster)

LAST_RESULTS = None

# per-block leaf dims: (A, B, C, D1, D2, D3)
BLOCKS = [
    (4, 7, 28, 16, 16, 16),
    (16, 16, 16, 16, 16, 16),
    (16, 16, 16, 16, 16, 16),
    (16, 16, 16, 16, 16, 16),
    (16, 16, 16, 4, 8, 8),
]


def _geom(b):
    A, B, C, D1, D2, D3 = BLOCKS[b]
    BC = B * C
    KC = (BC + P - 1) // P        # stage1 K tiles (zero-padded to P)
    ZA = ZC * A                   # stage1 moving free size (z-major)
    J = max(1, ZA // P)           # za partition tiles after transpose
    M = D2 * D3
    MC = (M + P - 1) // P         # m chunks
    MLO = min(P, M)
    M2 = ZJ * D1                  # stage2 lhsT free size
    OCT = ZC // ZJ                # output octet tiles (4)
    NO = OCT if J == 1 else 1     # octet-dependent U1D variants needed
    return A, B, C, D1, D2, D3, BC, KC, ZA, J, M, MC, MLO, NO, M2, OCT


def _qu(u1, b2, u2, b1, u3):
    """Host-composed stage weights: Q[r1, m, bc] and u1[a,d1,r1] (f32)."""
    u1, b2, u2, b1, u3 = [np.asarray(t, np.float64) for t in (u1, b2, u2, b1, u3)]
    g = np.einsum('pqr,rs->pqs', b2, b1)
    q = np.einsum('beq,cfs,pqs->pefbc', u2, u3, g, optimize=True)  # r1,d2,d3,b,c
    Rr, D2, D3, B, C = q.shape
    return (q.reshape(Rr, D2 * D3, B * C).astype(np.float32),
            u1.astype(np.float32))


def _pack_q(q, b):
    """Q[r1, m, bc] -> (P, KC, R, M): [p, kc, r1, m] = Q[r1, m, kc*P+p]."""
    A, B, C, D1, D2, D3, BC, KC, ZA, J, M, MC, MLO, NO, M2, OCT = _geom(b)
    out = np.zeros((P, KC, R, M), np.float32)
    qt = q.transpose(2, 0, 1)  # (bc, r1, m)
    for kc in range(KC):
        lo, hi = kc * P, min((kc + 1) * P, BC)
        out[: hi - lo, kc] = qt[lo:hi]
    return np.ascontiguousarray(out)


def _pack_u1d(u1, b):
    """u1[a,d1,r1] -> (P, R, NO, M2): [(z a), r1, o, (z8' d1)] = u1 * delta.

    The K dim is always the full 128 partitions of a tT tile (ZP z-values x A);
    the delta selects, for octet variant o, rows with z == o*ZJ + z8'.
    """
    A, B, C, D1, D2, D3, BC, KC, ZA, J, M, MC, MLO, NO, M2, OCT = _geom(b)
    ZP = P // A                   # z-values per tT partition tile
    out = np.zeros((ZP, A, R, NO, ZJ, D1), np.float32)
    u = u1.transpose(0, 2, 1)     # (a, r1, d1)
    for o in range(NO):
        for z8 in range(ZJ):
            out[o * ZJ + z8, :, :, o, z8, :] = u
    return np.ascontiguousarray(out.reshape(P, R, NO, M2))


def _pack_x(x_shard):
    """x (ZC, 784) -> (P, 2, ZA=128): [p, kc, z*4+a] = x[z, a, kc*P+p]."""
    A, B, C = BLOCKS[0][:3]
    BC = B * C
    xr = x_shard.reshape(ZC, A, BC).transpose(2, 0, 1).reshape(BC, ZC * A)
    out = np.zeros((P, 2, ZC * A), np.float32)
    out[:, 0] = xr[:P]
    out[: BC - P, 1] = xr[P:]
    return np.ascontiguousarray(out)


def _maybe_tf32(a):
    if DT_MODE != "tf32":
        return np.ascontiguousarray(a, np.float32)
    b = np.ascontiguousarray(a, np.float32).view(np.uint32)
    b = (b + np.uint32(0x1000)) & np.uint32(0xFFFFE000)
    return b.view(np.float32)


_NC_CACHE = {}


def _build():
    if DT_MODE in _NC_CACHE:
        return _NC_CACHE[DT_MODE]
    f32 = mybir.dt.float32
    mmdt = mybir.dt.float32r if DT_MODE == "tf32" else f32
    nc = bacc.Bacc("TRN2", num_devices=NCORES)

    x0 = nc.dram_tensor("x0", [P, 2, 128], mmdt, kind="ExternalInput")
    qs, us = [], []
    for b in range(5):
        A, B, C, D1, D2, D3, BC, KC, ZA, J, M, MC, MLO, NO, M2, OCT = _geom(b)
        qs.append(nc.dram_tensor(f"q{b}", [P, KC, R, M], mmdt, kind="ExternalInput"))
        us.append(nc.dram_tensor(f"u{b}", [P, R, NO, M2], mmdt, kind="ExternalInput"))
    y = nc.dram_tensor("y", [32, 4, 64], mmdt, kind="ExternalOutput")

    with TileContext(nc) as tc:
        with (
            tc.tile_pool(name="xs", bufs=2) as x_pool,
            tc.tile_pool(name="qp", bufs=1) as q_pool,
            tc.tile_pool(name="up", bufs=1) as u_pool,
            tc.tile_pool(name="tp", bufs=2) as t_pool,
            tc.tile_pool(name="ttp", bufs=1) as tt_pool,
            tc.tile_pool(name="yp", bufs=2) as y_pool,
            tc.tile_pool(name="idp", bufs=1) as id_pool,
            tc.tile_pool(name="ps1", bufs=2, space="PSUM") as ps1_pool,
            tc.tile_pool(name="psT", bufs=2, space="PSUM") as psT_pool,
            tc.tile_pool(name="ps2", bufs=2, space="PSUM") as ps2_pool,
        ):
            ident = id_pool.tile([P, P], mmdt)
            make_identity(nc, ident)
            relu = mybir.ActivationFunctionType.Relu

            u_sb = []
            for b in range(5):
                NOb, M2b = _geom(b)[13], _geom(b)[14]
                ut = u_pool.tile([P, R, NOb, M2b], mmdt, tag=f"u{b}")
                nc.sync.dma_start(out=ut, in_=us[b][:])
                u_sb.append(ut)

            act = x_pool.tile([P, 2, 128], mmdt, tag="act")
            nc.sync.dma_start(out=act, in_=x0[:])

            for b in range(5):
                (A, B, C, D1, D2, D3, BC, KC, ZA, J, M, MC, MLO,
                 NO, M2, OCT) = _geom(b)
                with nc.named_scope(f"block{b}"):
                    q_sb = q_pool.tile([P, KC, R, M], mmdt, tag="q")
                    nc.sync.dma_start(out=q_sb, in_=qs[b][:])

                    # ---- stage 1 + transpose, per r1 ----
                    tT = tt_pool.tile([P, J, R, M], mmdt, tag="tT")
                    for r1 in range(R):
                        tt_ = t_pool.tile([MLO, MC, ZA], mmdt, tag="t")
                        for mc in range(MC):
                            msl = slice(mc * MLO, (mc + 1) * MLO)
                            ps = ps1_pool.tile([MLO, ZA], f32)
                            for kc in range(KC):
                                nc.tensor.matmul(
                                    ps, q_sb[:, kc, r1, msl], act[:, kc],
                                    start=(kc == 0), stop=(kc == KC - 1),
                                )
                            nc.vector.tensor_copy(tt_[:, mc], ps)
                        for mc in range(MC):
                            msl = slice(mc * MLO, (mc + 1) * MLO)
                            pst = psT_pool.tile([P, J, MLO], mmdt, tag="pst")
                            for j in range(J):
                                src = tt_[:, mc, j * P:(j + 1) * P] if J > 1 \
                                    else tt_[:, mc, :]
                                nc.tensor.transpose(
                                    pst[:, j], src, ident[:MLO, :MLO]
                                )
                            nc.vector.tensor_copy(tT[:, :, r1, msl], pst)

                    # ---- stage 2: one matmul chain per z-octet ----
                    ysb = y_pool.tile([M2, OCT, M], mmdt, tag="y")
                    for o in range(OCT):
                        j, uo = (0, o) if J == 1 else (o, 0)
                        ps2 = ps2_pool.tile([M2, M], f32)
                        for r1 in range(R):
                            nc.tensor.matmul(
                                ps2,
                                u_sb[b][:, r1, uo, :],
                                tT[:, j, r1, :],
                                start=(r1 == 0), stop=(r1 == R - 1),
                            )
                        nc.scalar.activation(ysb[:, o], ps2, relu)

                    if b < 4:
                        # ---- transpose back to ((d2 d3) part, (z d1) free) ----
                        nact = x_pool.tile([P, MC, OCT * P], mmdt, tag="act")
                        for mc in range(MC):
                            msl = slice(mc * P, (mc + 1) * P)
                            psb = psT_pool.tile([P, OCT, P], mmdt, tag="psb")
                            for o in range(OCT):
                                nc.tensor.transpose(
                                    psb[:, o], ysb[:, o, msl], ident
                                )
                            nc.vector.tensor_copy(nact[:, mc], psb)
                        act = nact
                    else:
                        nc.sync.dma_start(out=y[:], in_=ysb)
    nc.compile()
    _NC_CACHE[DT_MODE] = nc
    return nc


def prep_inputs(x, cores):
    wq, wu = {}, {}
    for b in range(5):
        q, u1b = _qu(*cores[b])
        wq[b] = _maybe_tf32(_pack_q(q, b))
        wu[b] = _maybe_tf32(_pack_u1d(u1b, b))
    x = np.asarray(x, np.float32)
    in_maps = []
    for c in range(NCORES):
        m = {"x0": _maybe_tf32(_pack_x(x[c * ZC:(c + 1) * ZC]))}
        for b in range(5):
            m[f"q{b}"] = wq[b]
            m[f"u{b}"] = wu[b]
        in_maps.append(m)
    return in_maps


def assemble(per_core_y, w6, b6):
    """per_core_y[c]: (32=(z8 d1), 4=o, 64=m) -> (256, 10) output."""
    h = np.zeros((256, 256), np.float32)
    for c in range(NCORES):
        yc = np.asarray(per_core_y[c]).reshape(ZJ, 4, 4, 64)  # z8, d1, o, m
        h[:, c * ZC:(c + 1) * ZC] = yc.transpose(1, 3, 2, 0).reshape(256, ZC)
    out = h.T.astype(np.float64) @ np.asarray(w6, np.float64).T \
        + np.asarray(b6, np.float64)
    return out.astype(np.float32)


def kernel(x, u1, b2, u2, b1, u3,
           u1_2, b2_2, u2_2, b1_2, u3_2,
           u1_3, b2_3, u2_3, b1_3, u3_3,
           u1_4, b2_4, u2_4, b1_4, u3_4,
           u1_5, b2_5, u2_5, b1_5, u3_5,
           w6, b6):
    global LAST_RESULTS
    cores = [
        (u1, b2, u2, b1, u3),
        (u1_2, b2_2, u2_2, b1_2, u3_2),
        (u1_3, b2_3, u2_3, b1_3, u3_3),
        (u1_4, b2_4, u2_4, b1_4, u3_4),
        (u1_5, b2_5, u2_5, b1_5, u3_5),
    ]
    in_maps = prep_inputs(x, cores)
    nc = _build()
    res = run_bass_kernel_spmd(nc, in_maps, core_ids=list(range(NCORES)))
    LAST_RESULTS = res
    return assemble([res.results[c]["y"] for c in range(NCORES)], w6, b6)
